# revision 13
# baseline (speedup 1.0000x reference)
"""Causal self-attention (B=4, N=2048, D=1024, H=16) on 8 TRN2 NeuronCores.

Sharding: core c handles batch b = c//2 and head group g = c%2 (8 heads,
512 of the 1024 head dims). Each core computes
  qkv projection (its heads) -> causal attention -> partial out-projection
and returns oT_partial = (y_part @ W_out[:, cols].T).T  as [1024, 2048].
Host sums the two head-group partials per batch and transposes back.

All data is staged transposed on the host so every matmul contracts over
the SBUF partition axis:
  qkT = [Wq.T | Wk.T]-proj of xT     (scores need q/k with DH on partitions)
  v   = natural [n, d] layout, with a ones-column appended per head so the
        attention row-sum (softmax denominator) falls out of the same matmul.
Softmax runs without max-subtraction (scores ~ N(0,1), exp is safe in fp32),
masking is a 0/1 multiply on the 4 distinct diagonal-block patterns.
"""

import os
import sys

_TRN_REPO = "/opt/trn_rl_repo"
if _TRN_REPO not in sys.path:
    sys.path.insert(0, _TRN_REPO)

import numpy as np
import ml_dtypes
from contextlib import ExitStack

import concourse.bass as bass
import concourse.bacc as bacc
import concourse.tile as tile
from concourse import mybir
from concourse.bass_utils import run_bass_kernel_spmd

B, N, D, H, DH = 4, 2048, 1024, 16, 64
NCORES = 8
GH = 8          # heads per core
DL = GH * DH    # 512 local head dims
P = 128
CH = 512        # free-dim chunk (one PSUM bank of fp32)
NCH = N // CH   # 4
KT = D // P     # 8 contraction tiles for the projections

F32 = mybir.dt.float32
BF16 = mybir.dt.bfloat16

# dtype knobs (memory formats of the matmul operands)
X_DT = BF16     # xT tiles
W_DT = BF16     # wqk / wv / wo tiles
QK_DT = BF16    # qkT tiles (scores matmul operands)
V_DT = BF16     # v tiles
ATT_DT = BF16   # exp(S.T) tiles / mask
Y_DT = BF16     # yT tiles (out-projection rhs)

USE_GPSIMD_BCAST = True  # rank-1 PE broadcast by default

_NP_DT = {BF16: ml_dtypes.bfloat16, F32: np.float32}


def build_program() -> bass.Bass:
    nc = bacc.Bacc("TRN2", target_bir_lowering=False, debug=False)

    xT_d = nc.dram_tensor("xT", [D, N], X_DT, kind="ExternalInput").ap()
    wqk_d = nc.dram_tensor("wqk", [D, 2 * DL], W_DT, kind="ExternalInput").ap()
    wv_d = nc.dram_tensor("wv", [D, DL], W_DT, kind="ExternalInput").ap()
    wo_d = nc.dram_tensor("wo", [DL, D], W_DT, kind="ExternalInput").ap()
    mask_d = nc.dram_tensor("mask", [4 * P, CH], ATT_DT, kind="ExternalInput").ap()
    oT_d = nc.dram_tensor("oT", [D, N], F32, kind="ExternalOutput").ap()

    with tile.TileContext(nc) as tc, ExitStack() as ctx:
        def absorb(ps):
            # first-touch psum slot with a DVE op so slot-reuse waits land on
            # an instruction that allows multiple sem waits (MM allows one)
            nc.vector.memset(ps[0:1, 0:1], 0.0)

        xt_pool = ctx.enter_context(tc.tile_pool(name="xt", bufs=KT))
        wqk_pool = ctx.enter_context(tc.tile_pool(name="wqk", bufs=KT))
        wv_pool = ctx.enter_context(tc.tile_pool(name="wv", bufs=KT))
        qk_pool = ctx.enter_context(tc.tile_pool(name="qk", bufs=8))
        v_pool = ctx.enter_context(tc.tile_pool(name="v", bufs=N // P))
        mask_pool = ctx.enter_context(tc.tile_pool(name="mask", bufs=4))
        y_pool = ctx.enter_context(tc.tile_pool(name="y", bufs=4))
        wo_pool = ctx.enter_context(tc.tile_pool(name="wo", bufs=4))
        att_pool = ctx.enter_context(tc.tile_pool(name="att", bufs=4))
        nrm_pool = ctx.enter_context(tc.tile_pool(name="nrm", bufs=4))
        osb_pool = ctx.enter_context(tc.tile_pool(name="osb", bufs=4))

        # ---- loads -------------------------------------------------------
        xt = []
        wqkt = []
        wvt = []
        for k in range(KT):
            t = xt_pool.tile([P, N], X_DT)
            nc.sync.dma_start(t[:], xT_d[k * P:(k + 1) * P, :])
            xt.append(t)
            t = wqk_pool.tile([P, 2 * DL], W_DT)
            nc.sync.dma_start(t[:], wqk_d[k * P:(k + 1) * P, :])
            wqkt.append(t)
            t = wv_pool.tile([P, DL], W_DT)
            nc.sync.dma_start(t[:], wv_d[k * P:(k + 1) * P, :])
            wvt.append(t)
        mk = []
        wot = []
        for k in range(4):
            t = mask_pool.tile([P, CH], ATT_DT)
            nc.sync.dma_start(t[:], mask_d[k * P:(k + 1) * P, :])
            mk.append(t)
            t = wo_pool.tile([P, D], W_DT)
            nc.sync.dma_start(t[:], wo_d[k * P:(k + 1) * P, :])
            wot.append(t)

        ones64 = None
        if not USE_GPSIMD_BCAST:
            ones64 = nrm_pool.tile([1, 64], mybir.dt.float32r, tag="ones", bufs=1)
            nc.vector.memset(ones64[:], 1.0)

        # ---- phase 1: qkT = [WqT|WkT].T-proj, v = x @ WvT ---------------
        qkT = [qk_pool.tile([P, N], QK_DT, name=f"qkT{i}", tag="qkT") for i in range(8)]
        vsb = [v_pool.tile([P, GH * 65], V_DT, name=f"vsb{i}", tag="vsb") for i in range(N // P)]

        with tc.tile_pool(name="ps1", bufs=4, space="PSUM") as ps1:
            for t in range(8):
                for j in range(NCH):
                    ps = ps1.tile([P, CH], F32)
                    for k in range(KT):
                        nc.tensor.matmul(
                            ps[:],
                            lhsT=wqkt[k][:, t * P:(t + 1) * P],
                            rhs=xt[k][:, j * CH:(j + 1) * CH],
                            start=(k == 0),
                            stop=(k == KT - 1),
                        )
                    nc.vector.tensor_copy(qkT[t][:, j * CH:(j + 1) * CH], ps[:])
            for mt in range(N // P):
                ps = ps1.tile([P, DL], F32)
                for k in range(KT):
                    nc.tensor.matmul(
                        ps[:],
                        lhsT=xt[k][:, mt * P:(mt + 1) * P],
                        rhs=wvt[k][:],
                        start=(k == 0),
                        stop=(k == KT - 1),
                    )
                v3 = vsb[mt].rearrange("p (h c) -> p h c", c=65)
                nc.vector.tensor_copy(
                    v3[:, :, 0:64], ps.rearrange("p (h c) -> p h c", c=64)
                )
                nc.vector.memset(v3[:, :, 64:65], 1.0)

        # ---- phase 2: per-head causal attention -> yT -------------------
        yT = [y_pool.tile([P, N], Y_DT, name=f"yT{i}", tag="yT") for i in range(4)]

        with tc.tile_pool(name="ps2", bufs=1, space="PSUM") as ps2:
            for h in range(GH):
                tq, pq = h // 2, 64 * (h % 2)
                qsl = qkT[tq][pq:pq + 64, :]
                ksl = qkT[4 + tq][pq:pq + 64, :]
                vcol = 65 * h
                for j in range(NCH):
                    ops = ps2.tile([65, CH], F32, tag="out", bufs=2)
                    nmt = 4 * (j + 1)
                    for mt in range(nmt):
                        st = ps2.tile([P, CH], F32, tag="st", bufs=4)
                        nc.tensor.matmul(
                            st[:],
                            lhsT=ksl[:, mt * P:(mt + 1) * P],
                            rhs=qsl[:, j * CH:(j + 1) * CH],
                            start=True,
                            stop=True,
                        )
                        at = att_pool.tile([P, CH], ATT_DT)
                        ko = mt - 4 * j
                        nc.scalar.activation(
                            at[:], st[:], mybir.ActivationFunctionType.Exp
                        )
                        if ko >= 0:  # diagonal partial block
                            atm = att_pool.tile([P, CH], ATT_DT)
                            nc.vector.tensor_mul(atm[:], at[:], mk[ko][:])
                            at = atm
                        nc.tensor.matmul(
                            ops[:],
                            lhsT=vsb[mt][:, vcol:vcol + 65],
                            rhs=at[:],
                            start=(mt == 0),
                            stop=(mt == nmt - 1),
                        )
                    # normalize: yT[h dims, chunk j] = ops[0:64] / ops[64]
                    ysl = yT[tq][pq:pq + 64, j * CH:(j + 1) * CH]
                    if USE_GPSIMD_BCAST:
                        rc = nrm_pool.tile([1, CH], F32, tag="rc", bufs=2)
                        nc.vector.reciprocal(rc[:], ops[64:65, :])
                        bc = nrm_pool.tile([64, CH], F32, tag="bc", bufs=2)
                        nc.gpsimd.partition_broadcast(bc[:], rc[:])
                        nc.vector.tensor_mul(ysl, ops[0:64, :], bc[:])
                    else:
                        rc = nrm_pool.tile(
                            [1, CH], mybir.dt.float32r, tag="rc", bufs=2
                        )
                        with nc.allow_low_precision(reason="softmax recip bcast"):
                            nc.vector.reciprocal(rc[:], ops[64:65, :])
                        yu = nrm_pool.tile([64, CH], Y_DT, tag="yu", bufs=2)
                        nc.vector.tensor_copy(yu[:], ops[0:64, :])
                        bc = ps2.tile([64, CH], F32, tag="bc", bufs=2)
                        nc.tensor.matmul(
                            bc[:], lhsT=ones64[:], rhs=rc[:], start=True, stop=True
                        )
                        nc.vector.tensor_mul(ysl, yu[:], bc[:])

        # ---- phase 3: oT = (yT.T @ woT).T -------------------------------
        with tc.tile_pool(name="ps3", bufs=4, space="PSUM") as ps3:
            for e in range(8):
                for j in range(NCH):
                    ps = ps3.tile([P, CH], F32)
                    for d4 in range(4):
                        nc.tensor.matmul(
                            ps[:],
                            lhsT=wot[d4][:, e * P:(e + 1) * P],
                            rhs=yT[d4][:, j * CH:(j + 1) * CH],
                            start=(d4 == 0),
                            stop=(d4 == 3),
                        )
                    ob = osb_pool.tile([P, CH], F32)
                    nc.vector.tensor_copy(ob[:], ps[:])
                    nc.sync.dma_start(oT_d[e * P:(e + 1) * P, j * CH:(j + 1) * CH], ob[:])

    nc.compile()
    return nc


_PROGRAM = None


def _get_program() -> bass.Bass:
    global _PROGRAM
    if _PROGRAM is None:
        _PROGRAM = build_program()
    return _PROGRAM


def make_mask() -> np.ndarray:
    p = np.arange(P)[:, None]
    f = np.arange(CH)[None, :]
    m = np.zeros((4 * P, CH), np.float32)
    for k in range(4):
        m[k * P:(k + 1) * P] = (p + P * k <= f)
    return m


def make_in_maps(x, W_qkv, W_out):
    x = np.asarray(x, np.float32)
    W_qkv = np.asarray(W_qkv, np.float32)
    W_out = np.asarray(W_out, np.float32)
    mask = make_mask()
    xnp, wnp, anp = _NP_DT[X_DT], _NP_DT[W_DT], _NP_DT[ATT_DT]
    in_maps = []
    for c in range(NCORES):
        b, g = divmod(c, 2)
        wq = W_qkv[DL * g:DL * (g + 1)] * 0.125  # fold 1/sqrt(DH)
        wk = W_qkv[D + DL * g:D + DL * (g + 1)]
        wv = W_qkv[2 * D + DL * g:2 * D + DL * (g + 1)]
        in_maps.append({
            "xT": np.ascontiguousarray(x[b].T).astype(xnp),
            "wqk": np.ascontiguousarray(np.concatenate([wq, wk], 0).T).astype(wnp),
            "wv": np.ascontiguousarray(wv.T).astype(wnp),
            "wo": np.ascontiguousarray(W_out[:, DL * g:DL * (g + 1)].T).astype(wnp),
            "mask": mask.astype(anp),
        })
    return in_maps


def _assemble(results) -> np.ndarray:
    out = np.empty((B, N, D), np.float32)
    for b in range(B):
        out[b] = (results[2 * b]["oT"].astype(np.float32)
                  + results[2 * b + 1]["oT"].astype(np.float32)).T
    return out


def kernel(x, W_qkv, W_out) -> np.ndarray:
    nc = _get_program()
    in_maps = make_in_maps(x, W_qkv, W_out)
    res = run_bass_kernel_spmd(nc, in_maps, list(range(NCORES)))
    return _assemble(res.results)


def kernel_traced(x, W_qkv, W_out):
    """Like kernel() but with NTFF tracing; returns (out, BassKernelResults)."""
    nc = _get_program()
    in_maps = make_in_maps(x, W_qkv, W_out)
    res = run_bass_kernel_spmd(nc, in_maps, list(range(NCORES)), trace=True)
    return _assemble(res.results), res


def kernel_timed(x, W_qkv, W_out, iters=10):
    """Run on HW repeatedly with device-resident inputs; returns
    (out, per_call_seconds_list). Mirrors bass2jax.run_bass_via_pjrt's
    multi-core path but keeps the jitted callable for re-dispatch."""
    import jax
    import numpy as _np
    from jax.sharding import Mesh, PartitionSpec
    from jax.experimental.shard_map import shard_map
    from concourse import bass2jax, mybir as _mb
    import time as _time

    bass2jax.install_neuronx_cc_hook()
    nc = _get_program()
    in_maps = make_in_maps(x, W_qkv, W_out)

    part_name = nc.partition_id_tensor.name if nc.partition_id_tensor else None
    in_names, out_names, out_avals, zero_outs = [], [], [], []
    for alloc in nc.m.functions[0].allocations:
        if not isinstance(alloc, _mb.MemoryLocationSet):
            continue
        name = alloc.memorylocations[0].name
        if alloc.kind == "ExternalInput":
            if name != part_name:
                in_names.append(name)
        elif alloc.kind == "ExternalOutput":
            out_names.append(name)
            shape = tuple(alloc.tensor_shape)
            dtype = _mb.dt.np(alloc.dtype)
            out_avals.append(jax.core.ShapedArray(shape, dtype))
            zero_outs.append(_np.zeros(shape, dtype))
    n_params = len(in_names)
    all_names = in_names + out_names
    if part_name is not None:
        all_names = all_names + [part_name]

    def _body(*args):
        operands = list(args)
        if part_name is not None:
            operands.append(bass2jax.partition_id_tensor())
        outs = bass2jax._bass_exec_p.bind(
            *operands,
            out_avals=tuple(out_avals),
            in_names=tuple(all_names),
            out_names=tuple(out_names),
            lowering_input_output_aliases=(),
            sim_require_finite=True,
            sim_require_nnan=True,
            nc=nc,
        )
        return tuple(outs)

    devices = jax.devices()[:NCORES]
    mesh = Mesh(_np.asarray(devices), ("core",))
    nin = n_params + len(out_names)
    fn = jax.jit(
        shard_map(
            _body,
            mesh=mesh,
            in_specs=(PartitionSpec("core"),) * nin,
            out_specs=(PartitionSpec("core"),) * len(out_names),
            check_rep=False,
        ),
        keep_unused=True,
    )
    concat_in = [
        _np.concatenate([_np.asarray(in_maps[c][nm]) for c in range(NCORES)], axis=0)
        for nm in in_names
    ] + [
        _np.zeros((NCORES * z.shape[0], *z.shape[1:]), z.dtype) for z in zero_outs
    ]
    dev_in = [jax.device_put(a) for a in concat_in]
    out = fn(*dev_in)  # compile + warm
    jax.block_until_ready(out)
    times = []
    for _ in range(iters):
        t0 = _time.perf_counter()
        out = fn(*dev_in)
        jax.block_until_ready(out)
        times.append(_time.perf_counter() - t0)
    results = [
        {nm: _np.asarray(out[i]).reshape(NCORES, *out_avals[i].shape)[c]
         for i, nm in enumerate(out_names)}
        for c in range(NCORES)
    ]
    return _assemble(results), times


# revision 15
# speedup vs baseline: 195.5603x; 195.5603x over previous
"""Causal self-attention (B=4, N=2048, D=1024, H=16) on 8 TRN2 NeuronCores.

Sharding: core c handles batch b = c//2 and head group g = c%2 (8 heads,
512 of the 1024 head dims). Each core computes
  qkv projection (its heads) -> causal attention -> partial out-projection
and returns oT_partial = (y_part @ W_out[:, cols].T).T  as [1024, 2048].
Host sums the two head-group partials per batch and transposes back.

All data is staged transposed on the host so every matmul contracts over
the SBUF partition axis:
  qkT = [Wq.T | Wk.T]-proj of xT     (scores need q/k with DH on partitions)
  v   = natural [n, d] layout, with a ones-column appended per head so the
        attention row-sum (softmax denominator) falls out of the same matmul.
Softmax runs without max-subtraction (scores ~ N(0,1), exp is safe in fp32),
masking is a 0/1 multiply on the 4 distinct diagonal-block patterns.
"""

import os
import sys

_TRN_REPO = "/opt/trn_rl_repo"
if _TRN_REPO not in sys.path:
    sys.path.insert(0, _TRN_REPO)

import numpy as np
import ml_dtypes
from contextlib import ExitStack

import concourse.bass as bass
import concourse.bacc as bacc
import concourse.tile as tile
from concourse import mybir
from concourse.bass_utils import run_bass_kernel_spmd

B, N, D, H, DH = 4, 2048, 1024, 16, 64
NCORES = 8
GH = 8          # heads per core
DL = GH * DH    # 512 local head dims
P = 128
CH = 512        # free-dim chunk (one PSUM bank of fp32)
NCH = N // CH   # 4
KT = D // P     # 8 contraction tiles for the projections

F32 = mybir.dt.float32
BF16 = mybir.dt.bfloat16

# dtype knobs (memory formats of the matmul operands)
X_DT = BF16     # xT tiles
W_DT = BF16     # wqk / wv / wo tiles
QK_DT = BF16    # qkT tiles (scores matmul operands)
V_DT = BF16     # v tiles
ATT_DT = BF16   # exp(S.T) tiles / mask
Y_DT = BF16     # yT tiles (out-projection rhs)

USE_GPSIMD_BCAST = True  # rank-1 PE broadcast by default

_NP_DT = {BF16: ml_dtypes.bfloat16, F32: np.float32}


def build_program(reps: int = 1) -> bass.Bass:
    nc = bacc.Bacc("TRN2", target_bir_lowering=False, debug=False)

    xT_d = nc.dram_tensor("xT", [D, N], X_DT, kind="ExternalInput").ap()
    wqk_d = nc.dram_tensor("wqk", [D, 2 * DL], W_DT, kind="ExternalInput").ap()
    wv_d = nc.dram_tensor("wv", [D, DL], W_DT, kind="ExternalInput").ap()
    wo_d = nc.dram_tensor("wo", [DL, D], W_DT, kind="ExternalInput").ap()
    mask_d = nc.dram_tensor("mask", [4 * P, CH], ATT_DT, kind="ExternalInput").ap()
    oT_d = nc.dram_tensor("oT", [D, N], F32, kind="ExternalOutput").ap()

    with tile.TileContext(nc) as tc:
        for rep in range(reps):
            _emit_rep(nc, tc, rep, xT_d, wqk_d, wv_d, wo_d, mask_d, oT_d)

    nc.compile()
    return nc


def _emit_rep(nc, tc, rep, xT_d, wqk_d, wv_d, wo_d, mask_d, oT_d):
    r = f"_r{rep}"
    with ExitStack() as ctx:
        xt_pool = ctx.enter_context(tc.tile_pool(name="xt" + r, bufs=KT))
        wqk_pool = ctx.enter_context(tc.tile_pool(name="wqk" + r, bufs=KT))
        wv_pool = ctx.enter_context(tc.tile_pool(name="wv" + r, bufs=KT))
        qk_pool = ctx.enter_context(tc.tile_pool(name="qk" + r, bufs=8))
        v_pool = ctx.enter_context(tc.tile_pool(name="v" + r, bufs=N // P))
        mask_pool = ctx.enter_context(tc.tile_pool(name="mask" + r, bufs=4))
        y_pool = ctx.enter_context(tc.tile_pool(name="y" + r, bufs=4))
        wo_pool = ctx.enter_context(tc.tile_pool(name="wo" + r, bufs=4))
        att_pool = ctx.enter_context(tc.tile_pool(name="att" + r, bufs=4))
        nrm_pool = ctx.enter_context(tc.tile_pool(name="nrm" + r, bufs=4))
        osb_pool = ctx.enter_context(tc.tile_pool(name="osb" + r, bufs=4))

        # ---- loads -------------------------------------------------------
        xt = []
        wqkt = []
        wvt = []
        for k in range(KT):
            t = xt_pool.tile([P, N], X_DT, name=f"xt{k}{r}", tag="xt")
            nc.sync.dma_start(t[:], xT_d[k * P:(k + 1) * P, :])
            xt.append(t)
            t = wqk_pool.tile([P, 2 * DL], W_DT, name=f"wqkt{k}{r}", tag="wqkt")
            nc.sync.dma_start(t[:], wqk_d[k * P:(k + 1) * P, :])
            wqkt.append(t)
            t = wv_pool.tile([P, DL], W_DT, name=f"wvt{k}{r}", tag="wvt")
            nc.sync.dma_start(t[:], wv_d[k * P:(k + 1) * P, :])
            wvt.append(t)
        mk = []
        wot = []
        for k in range(4):
            t = mask_pool.tile([P, CH], ATT_DT, name=f"mk{k}{r}", tag="mk")
            nc.sync.dma_start(t[:], mask_d[k * P:(k + 1) * P, :])
            mk.append(t)
            t = wo_pool.tile([P, D], W_DT, name=f"wot{k}{r}", tag="wot")
            nc.sync.dma_start(t[:], wo_d[k * P:(k + 1) * P, :])
            wot.append(t)

        # ---- phase 1: qkT = [WqT|WkT].T-proj, v = x @ WvT ---------------
        qkT = [qk_pool.tile([P, N], QK_DT, name=f"qkT{i}{r}", tag="qkT")
               for i in range(8)]
        vsb = [v_pool.tile([P, GH * 65], V_DT, name=f"vsb{i}{r}", tag="vsb")
               for i in range(N // P)]

        with tc.tile_pool(name="ps1" + r, bufs=4, space="PSUM") as ps1:
            for t in range(8):
                for j in range(NCH):
                    ps = ps1.tile([P, CH], F32, name=f"p1_{t}_{j}{r}", tag="p1")
                    for k in range(KT):
                        nc.tensor.matmul(
                            ps[:],
                            lhsT=wqkt[k][:, t * P:(t + 1) * P],
                            rhs=xt[k][:, j * CH:(j + 1) * CH],
                            start=(k == 0),
                            stop=(k == KT - 1),
                        )
                    nc.vector.tensor_copy(qkT[t][:, j * CH:(j + 1) * CH], ps[:])
            for mt in range(N // P):
                ps = ps1.tile([P, DL], F32, name=f"p1v_{mt}{r}", tag="p1")
                for k in range(KT):
                    nc.tensor.matmul(
                        ps[:],
                        lhsT=xt[k][:, mt * P:(mt + 1) * P],
                        rhs=wvt[k][:],
                        start=(k == 0),
                        stop=(k == KT - 1),
                    )
                v3 = vsb[mt].rearrange("p (h c) -> p h c", c=65)
                nc.vector.tensor_copy(
                    v3[:, :, 0:64], ps.rearrange("p (h c) -> p h c", c=64)
                )
                nc.vector.memset(v3[:, :, 64:65], 1.0)

        # ---- phase 2: per-head causal attention -> yT -------------------
        yT = [y_pool.tile([P, N], Y_DT, name=f"yT{i}{r}", tag="yT")
              for i in range(4)]

        with tc.tile_pool(name="ps2" + r, bufs=1, space="PSUM") as ps2:
            for h in range(GH):
                tq, pq = h // 2, 64 * (h % 2)
                qsl = qkT[tq][pq:pq + 64, :]
                ksl = qkT[4 + tq][pq:pq + 64, :]
                vcol = 65 * h
                for j in range(NCH):
                    ops = ps2.tile([65, CH], F32, name=f"ops_{h}_{j}{r}",
                                   tag="out", bufs=2)
                    nmt = 4 * (j + 1)
                    for mt in range(nmt):
                        st = ps2.tile([P, CH], F32, name=f"st_{h}_{j}_{mt}{r}",
                                      tag="st", bufs=4)
                        nc.tensor.matmul(
                            st[:],
                            lhsT=ksl[:, mt * P:(mt + 1) * P],
                            rhs=qsl[:, j * CH:(j + 1) * CH],
                            start=True,
                            stop=True,
                        )
                        at = att_pool.tile([P, CH], ATT_DT,
                                           name=f"at_{h}_{j}_{mt}{r}", tag="at")
                        ko = mt - 4 * j
                        nc.scalar.activation(
                            at[:], st[:], mybir.ActivationFunctionType.Exp
                        )
                        if ko >= 0:  # diagonal partial block
                            atm = att_pool.tile([P, CH], ATT_DT,
                                                name=f"atm_{h}_{j}_{mt}{r}", tag="at")
                            nc.vector.tensor_mul(atm[:], at[:], mk[ko][:])
                            at = atm
                        nc.tensor.matmul(
                            ops[:],
                            lhsT=vsb[mt][:, vcol:vcol + 65],
                            rhs=at[:],
                            start=(mt == 0),
                            stop=(mt == nmt - 1),
                        )
                    # normalize: yT[h dims, chunk j] = ops[0:64] / ops[64]
                    ysl = yT[tq][pq:pq + 64, j * CH:(j + 1) * CH]
                    rc = nrm_pool.tile([1, CH], F32, name=f"rc_{h}_{j}{r}",
                                       tag="rc", bufs=2)
                    nc.vector.reciprocal(rc[:], ops[64:65, :])
                    bc = nrm_pool.tile([64, CH], F32, name=f"bc_{h}_{j}{r}",
                                       tag="bc", bufs=2)
                    nc.gpsimd.partition_broadcast(bc[:], rc[:])
                    nc.vector.tensor_mul(ysl, ops[0:64, :], bc[:])

        # ---- phase 3: oT = (yT.T @ woT).T -------------------------------
        with tc.tile_pool(name="ps3" + r, bufs=4, space="PSUM") as ps3:
            for e in range(8):
                for j in range(NCH):
                    ps = ps3.tile([P, CH], F32, name=f"p3_{e}_{j}{r}", tag="p3")
                    for d4 in range(4):
                        nc.tensor.matmul(
                            ps[:],
                            lhsT=wot[d4][:, e * P:(e + 1) * P],
                            rhs=yT[d4][:, j * CH:(j + 1) * CH],
                            start=(d4 == 0),
                            stop=(d4 == 3),
                        )
                    ob = osb_pool.tile([P, CH], F32, name=f"ob_{e}_{j}{r}", tag="ob")
                    nc.vector.tensor_copy(ob[:], ps[:])
                    nc.sync.dma_start(
                        oT_d[e * P:(e + 1) * P, j * CH:(j + 1) * CH], ob[:]
                    )


_PROGRAM = None


def _get_program() -> bass.Bass:
    global _PROGRAM
    if _PROGRAM is None:
        _PROGRAM = build_program()
    return _PROGRAM


def _timing_program(reps: int) -> bass.Bass:
    return build_program(reps=reps)


def make_mask() -> np.ndarray:
    p = np.arange(P)[:, None]
    f = np.arange(CH)[None, :]
    m = np.zeros((4 * P, CH), np.float32)
    for k in range(4):
        m[k * P:(k + 1) * P] = (p + P * k <= f)
    return m


def make_in_maps(x, W_qkv, W_out):
    x = np.asarray(x, np.float32)
    W_qkv = np.asarray(W_qkv, np.float32)
    W_out = np.asarray(W_out, np.float32)
    mask = make_mask()
    xnp, wnp, anp = _NP_DT[X_DT], _NP_DT[W_DT], _NP_DT[ATT_DT]
    in_maps = []
    for c in range(NCORES):
        b, g = divmod(c, 2)
        wq = W_qkv[DL * g:DL * (g + 1)] * 0.125  # fold 1/sqrt(DH)
        wk = W_qkv[D + DL * g:D + DL * (g + 1)]
        wv = W_qkv[2 * D + DL * g:2 * D + DL * (g + 1)]
        in_maps.append({
            "xT": np.ascontiguousarray(x[b].T).astype(xnp),
            "wqk": np.ascontiguousarray(np.concatenate([wq, wk], 0).T).astype(wnp),
            "wv": np.ascontiguousarray(wv.T).astype(wnp),
            "wo": np.ascontiguousarray(W_out[:, DL * g:DL * (g + 1)].T).astype(wnp),
            "mask": mask.astype(anp),
        })
    return in_maps


def _assemble(results) -> np.ndarray:
    out = np.empty((B, N, D), np.float32)
    for b in range(B):
        out[b] = (results[2 * b]["oT"].astype(np.float32)
                  + results[2 * b + 1]["oT"].astype(np.float32)).T
    return out


def kernel(x, W_qkv, W_out) -> np.ndarray:
    nc = _get_program()
    in_maps = make_in_maps(x, W_qkv, W_out)
    res = run_bass_kernel_spmd(nc, in_maps, list(range(NCORES)))
    return _assemble(res.results)


def kernel_traced(x, W_qkv, W_out):
    """Like kernel() but with NTFF tracing; returns (out, BassKernelResults)."""
    nc = _get_program()
    in_maps = make_in_maps(x, W_qkv, W_out)
    res = run_bass_kernel_spmd(nc, in_maps, list(range(NCORES)), trace=True)
    return _assemble(res.results), res


def kernel_timed(x, W_qkv, W_out, iters=10, nc=None):
    """Run on HW repeatedly with device-resident inputs; returns
    (out, per_call_seconds_list). Mirrors bass2jax.run_bass_via_pjrt's
    multi-core path but keeps the jitted callable for re-dispatch."""
    import jax
    import numpy as _np
    from jax.sharding import Mesh, PartitionSpec
    from jax.experimental.shard_map import shard_map
    from concourse import bass2jax, mybir as _mb
    import time as _time

    bass2jax.install_neuronx_cc_hook()
    if nc is None:
        nc = _get_program()
    in_maps = make_in_maps(x, W_qkv, W_out)

    part_name = nc.partition_id_tensor.name if nc.partition_id_tensor else None
    in_names, out_names, out_avals, zero_outs = [], [], [], []
    for alloc in nc.m.functions[0].allocations:
        if not isinstance(alloc, _mb.MemoryLocationSet):
            continue
        name = alloc.memorylocations[0].name
        if alloc.kind == "ExternalInput":
            if name != part_name:
                in_names.append(name)
        elif alloc.kind == "ExternalOutput":
            out_names.append(name)
            shape = tuple(alloc.tensor_shape)
            dtype = _mb.dt.np(alloc.dtype)
            out_avals.append(jax.core.ShapedArray(shape, dtype))
            zero_outs.append(_np.zeros(shape, dtype))
    n_params = len(in_names)
    all_names = in_names + out_names
    if part_name is not None:
        all_names = all_names + [part_name]

    def _body(*args):
        operands = list(args)
        if part_name is not None:
            operands.append(bass2jax.partition_id_tensor())
        outs = bass2jax._bass_exec_p.bind(
            *operands,
            out_avals=tuple(out_avals),
            in_names=tuple(all_names),
            out_names=tuple(out_names),
            lowering_input_output_aliases=(),
            sim_require_finite=True,
            sim_require_nnan=True,
            nc=nc,
        )
        return tuple(outs)

    devices = jax.devices()[:NCORES]
    mesh = Mesh(_np.asarray(devices), ("core",))
    nin = n_params + len(out_names)
    fn = jax.jit(
        shard_map(
            _body,
            mesh=mesh,
            in_specs=(PartitionSpec("core"),) * nin,
            out_specs=(PartitionSpec("core"),) * len(out_names),
            check_rep=False,
        ),
        keep_unused=True,
    )
    concat_in = [
        _np.concatenate([_np.asarray(in_maps[c][nm]) for c in range(NCORES)], axis=0)
        for nm in in_names
    ] + [
        _np.zeros((NCORES * z.shape[0], *z.shape[1:]), z.dtype) for z in zero_outs
    ]
    dev_in = [jax.device_put(a) for a in concat_in]
    out = fn(*dev_in)  # compile + warm
    jax.block_until_ready(out)
    times = []
    for _ in range(iters):
        t0 = _time.perf_counter()
        out = fn(*dev_in)
        jax.block_until_ready(out)
        times.append(_time.perf_counter() - t0)
    results = [
        {nm: _np.asarray(out[i]).reshape(NCORES, *out_avals[i].shape)[c]
         for i, nm in enumerate(out_names)}
        for c in range(NCORES)
    ]
    return _assemble(results), times


# revision 20
# speedup vs baseline: 293.6928x; 1.5018x over previous
"""Causal self-attention (B=4, N=2048, D=1024, H=16) on 8 TRN2 NeuronCores.

Sharding: core c handles batch b = c//2 and head group g = c%2 (8 heads,
512 of the 1024 head dims). Each core computes
  qkv projection (its heads) -> causal attention -> partial out-projection
and returns oT_partial = (y_part @ W_out[:, cols].T).T  as [1024, 2048].
Host sums the two head-group partials per batch and transposes back.

All data is staged transposed on the host so every matmul contracts over
the SBUF partition axis:
  qkT = [Wq.T | Wk.T]-proj of xT     (scores need q/k with DH on partitions)
  v   = natural [n, d] layout, with a ones-column appended per head so the
        attention row-sum (softmax denominator) falls out of the same matmul.
Softmax runs without max-subtraction (scores ~ N(0,1), exp is safe in fp32),
masking is a 0/1 multiply on the 4 distinct diagonal-block patterns.
"""

import os
import sys

_TRN_REPO = "/opt/trn_rl_repo"
if _TRN_REPO not in sys.path:
    sys.path.insert(0, _TRN_REPO)

import numpy as np
import ml_dtypes
from contextlib import ExitStack

import concourse.bass as bass
import concourse.bacc as bacc
import concourse.tile as tile
from concourse import mybir
from concourse.bass_utils import run_bass_kernel_spmd

B, N, D, H, DH = 4, 2048, 1024, 16, 64
NCORES = 8
GH = 8          # heads per core
DL = GH * DH    # 512 local head dims
P = 128
CH = 512        # free-dim chunk (one PSUM bank of fp32)
NCH = N // CH   # 4
KT = D // P     # 8 contraction tiles for the projections

F32 = mybir.dt.float32
BF16 = mybir.dt.bfloat16

# dtype knobs (memory formats of the matmul operands)
X_DT = BF16     # xT tiles
W_DT = BF16     # wqk / wv / wo tiles
QK_DT = BF16    # qkT tiles (scores matmul operands)
V_DT = BF16     # v tiles
ATT_DT = BF16   # exp(S.T) tiles / mask
Y_DT = BF16     # yT tiles (out-projection rhs)

USE_GPSIMD_BCAST = True  # rank-1 PE broadcast by default

_NP_DT = {BF16: ml_dtypes.bfloat16, F32: np.float32}


def build_program(reps: int = 1) -> bass.Bass:
    nc = bacc.Bacc("TRN2", target_bir_lowering=False, debug=False)

    xT_d = nc.dram_tensor("xT", [D, N], X_DT, kind="ExternalInput").ap()
    wqk_d = nc.dram_tensor("wqk", [D, 2 * DL], W_DT, kind="ExternalInput").ap()
    wv_d = nc.dram_tensor("wv", [D, DL], W_DT, kind="ExternalInput").ap()
    wo_d = nc.dram_tensor("wo", [DL, D], W_DT, kind="ExternalInput").ap()
    mask_d = nc.dram_tensor("mask", [4 * P, 2 * CH], ATT_DT, kind="ExternalInput").ap()
    oT_d = nc.dram_tensor("oT", [D, N], F32, kind="ExternalOutput").ap()

    with tile.TileContext(nc) as tc:
        for rep in range(reps):
            _emit_rep(nc, tc, rep, xT_d, wqk_d, wv_d, wo_d, mask_d, oT_d)

    nc.compile()
    return nc


def _emit_rep(nc, tc, rep, xT_d, wqk_d, wv_d, wo_d, mask_d, oT_d):
    r = f"_r{rep}"
    with ExitStack() as ctx:
        xt_pool = ctx.enter_context(tc.tile_pool(name="xt" + r, bufs=KT))
        wqk_pool = ctx.enter_context(tc.tile_pool(name="wqk" + r, bufs=KT))
        wv_pool = ctx.enter_context(tc.tile_pool(name="wv" + r, bufs=KT))
        qk_pool = ctx.enter_context(tc.tile_pool(name="qk" + r, bufs=8))
        v_pool = ctx.enter_context(tc.tile_pool(name="v" + r, bufs=N // P))
        mask_pool = ctx.enter_context(tc.tile_pool(name="mask" + r, bufs=4))
        y_pool = ctx.enter_context(tc.tile_pool(name="y" + r, bufs=4))
        wo_pool = ctx.enter_context(tc.tile_pool(name="wo" + r, bufs=4))
        att_pool = ctx.enter_context(tc.tile_pool(name="att" + r, bufs=4))
        nrm_pool = ctx.enter_context(tc.tile_pool(name="nrm" + r, bufs=4))
        osb_pool = ctx.enter_context(tc.tile_pool(name="osb" + r, bufs=4))

        # ---- loads -------------------------------------------------------
        xt = []
        wqkt = []
        wvt = []
        for k in range(KT):
            t = xt_pool.tile([P, N], X_DT, name=f"xt{k}{r}", tag="xt")
            nc.sync.dma_start(t[:], xT_d[k * P:(k + 1) * P, :])
            xt.append(t)
            t = wqk_pool.tile([P, 2 * DL], W_DT, name=f"wqkt{k}{r}", tag="wqkt")
            nc.sync.dma_start(t[:], wqk_d[k * P:(k + 1) * P, :])
            wqkt.append(t)
            t = wv_pool.tile([P, DL], W_DT, name=f"wvt{k}{r}", tag="wvt")
            nc.sync.dma_start(t[:], wv_d[k * P:(k + 1) * P, :])
            wvt.append(t)
        mk = []
        wot = []
        for k in range(4):
            t = mask_pool.tile([P, 2 * CH], ATT_DT, name=f"mk{k}{r}", tag="mk")
            nc.sync.dma_start(t[:], mask_d[k * P:(k + 1) * P, :])
            mk.append(t)
            t = wo_pool.tile([P, D], W_DT, name=f"wot{k}{r}", tag="wot")
            nc.sync.dma_start(t[:], wo_d[k * P:(k + 1) * P, :])
            wot.append(t)

        # ---- phase 1: qkT = [WqT|WkT].T-proj, v = x @ WvT ---------------
        qkT = [qk_pool.tile([P, N], QK_DT, name=f"qkT{i}{r}", tag="qkT")
               for i in range(8)]
        vsb = [v_pool.tile([P, GH * 65], V_DT, name=f"vsb{i}{r}", tag="vsb")
               for i in range(N // P)]

        with tc.tile_pool(name="ps1" + r, bufs=4, space="PSUM") as ps1:
            for t in range(8):
                for j in range(NCH):
                    ps = ps1.tile([P, CH], F32, name=f"p1_{t}_{j}{r}", tag="p1")
                    for k in range(KT):
                        nc.tensor.matmul(
                            ps[:],
                            lhsT=wqkt[k][:, t * P:(t + 1) * P],
                            rhs=xt[k][:, j * CH:(j + 1) * CH],
                            start=(k == 0),
                            stop=(k == KT - 1),
                        )
                    nc.vector.tensor_copy(qkT[t][:, j * CH:(j + 1) * CH], ps[:])
            for mt in range(N // P):
                ps = ps1.tile([P, DL], F32, name=f"p1v_{mt}{r}", tag="p1")
                for k in range(KT):
                    nc.tensor.matmul(
                        ps[:],
                        lhsT=xt[k][:, mt * P:(mt + 1) * P],
                        rhs=wvt[k][:],
                        start=(k == 0),
                        stop=(k == KT - 1),
                    )
                v3 = vsb[mt].rearrange("p (h c) -> p h c", c=65)
                nc.vector.tensor_copy(
                    v3[:, :, 0:64], ps.rearrange("p (h c) -> p h c", c=64)
                )
                nc.vector.memset(v3[:, :, 64:65], 1.0)

        # ---- phase 2: causal attention, head pairs row-tiled -> yT ------
        yT = [y_pool.tile([P, N], Y_DT, name=f"yT{i}{r}", tag="yT")
              for i in range(4)]

        with tc.tile_pool(name="ps2" + r, bufs=1, space="PSUM") as ps2:
            for hp in range(4):
                h0, h1 = 2 * hp, 2 * hp + 1
                qt, kt = qkT[hp], qkT[4 + hp]
                for j in range(NCH):
                    ops0 = ps2.tile([65, CH], F32, name=f"ops0_{hp}_{j}{r}",
                                    tag="out0", bufs=2)
                    ops1 = ps2.tile([65, CH], F32, name=f"ops1_{hp}_{j}{r}",
                                    tag="out1", bufs=2)
                    nmt = 4 * (j + 1)
                    for mt in range(nmt):
                        st = ps2.tile([P, 2 * CH], F32, name=f"st_{hp}_{j}_{mt}{r}",
                                      tag="st", bufs=2)
                        # both heads' score blocks concurrently (row strips)
                        nc.tensor.matmul(
                            st[:, 0:CH],
                            lhsT=kt[0:64, mt * P:(mt + 1) * P],
                            rhs=qt[0:64, j * CH:(j + 1) * CH],
                            start=True,
                            stop=True,
                        )
                        nc.tensor.matmul(
                            st[:, CH:2 * CH],
                            lhsT=kt[64:128, mt * P:(mt + 1) * P],
                            rhs=qt[64:128, j * CH:(j + 1) * CH],
                            start=True,
                            stop=True,
                        )
                        at = att_pool.tile([P, 2 * CH], ATT_DT,
                                           name=f"at_{hp}_{j}_{mt}{r}", tag="at")
                        nc.scalar.activation(
                            at[:], st[:], mybir.ActivationFunctionType.Exp
                        )
                        ko = mt - 4 * j
                        if ko >= 0:  # diagonal partial block (same mask both heads)
                            atm = att_pool.tile([P, 2 * CH], ATT_DT,
                                                name=f"atm_{hp}_{j}_{mt}{r}", tag="at")
                            nc.vector.tensor_mul(atm[:], at[:], mk[ko][:])
                            at = atm
                        nc.tensor.matmul(
                            ops0[:],
                            lhsT=vsb[mt][:, 65 * h0:65 * h0 + 65],
                            rhs=at[:, 0:CH],
                            start=(mt == 0),
                            stop=(mt == nmt - 1),
                        )
                        nc.tensor.matmul(
                            ops1[:],
                            lhsT=vsb[mt][:, 65 * h1:65 * h1 + 65],
                            rhs=at[:, CH:2 * CH],
                            start=(mt == 0),
                            stop=(mt == nmt - 1),
                        )
                    # normalize while copying out of PSUM:
                    # yT rows = ops[0:64] * bcast(1 / ops[64])
                    jc = slice(j * CH, (j + 1) * CH)
                    for ii, ops in ((0, ops0), (1, ops1)):
                        dn = nrm_pool.tile([1, CH], F32,
                                           name=f"dn{ii}_{hp}_{j}{r}", tag="dn",
                                           bufs=4)
                        nc.vector.tensor_copy(dn[:], ops[64:65, :])
                        rc = nrm_pool.tile([1, CH], F32,
                                           name=f"rc{ii}_{hp}_{j}{r}", tag="rc",
                                           bufs=4)
                        nc.vector.reciprocal_approx_fast(out=rc[:], in_=dn[:])
                        bc = nrm_pool.tile([64, CH], F32,
                                           name=f"bc{ii}_{hp}_{j}{r}", tag="bc",
                                           bufs=4)
                        nc.gpsimd.partition_broadcast(bc[:], rc[:])
                        nc.vector.tensor_mul(
                            yT[hp][64 * ii:64 * ii + 64, jc],
                            ops[0:64, :],
                            bc[:],
                        )

        # ---- phase 3: oT = (yT.T @ woT).T -------------------------------
        with tc.tile_pool(name="ps3" + r, bufs=4, space="PSUM") as ps3:
            for e in range(8):
                for j in range(NCH):
                    ps = ps3.tile([P, CH], F32, name=f"p3_{e}_{j}{r}", tag="p3")
                    for d4 in range(4):
                        nc.tensor.matmul(
                            ps[:],
                            lhsT=wot[d4][:, e * P:(e + 1) * P],
                            rhs=yT[d4][:, j * CH:(j + 1) * CH],
                            start=(d4 == 0),
                            stop=(d4 == 3),
                        )
                    ob = osb_pool.tile([P, CH], F32, name=f"ob_{e}_{j}{r}", tag="ob")
                    nc.vector.tensor_copy(ob[:], ps[:])
                    nc.sync.dma_start(
                        oT_d[e * P:(e + 1) * P, j * CH:(j + 1) * CH], ob[:]
                    )


_PROGRAM = None


def _get_program() -> bass.Bass:
    global _PROGRAM
    if _PROGRAM is None:
        _PROGRAM = build_program()
    return _PROGRAM


def _timing_program(reps: int) -> bass.Bass:
    return build_program(reps=reps)


def make_mask() -> np.ndarray:
    p = np.arange(P)[:, None]
    f = np.arange(CH)[None, :]
    m = np.zeros((4 * P, CH), np.float32)
    for k in range(4):
        m[k * P:(k + 1) * P] = (p + P * k <= f)
    return np.tile(m, (1, 2))


def make_in_maps(x, W_qkv, W_out):
    x = np.asarray(x, np.float32)
    W_qkv = np.asarray(W_qkv, np.float32)
    W_out = np.asarray(W_out, np.float32)
    mask = make_mask()
    xnp, wnp, anp = _NP_DT[X_DT], _NP_DT[W_DT], _NP_DT[ATT_DT]
    in_maps = []
    for c in range(NCORES):
        b, g = divmod(c, 2)
        wq = W_qkv[DL * g:DL * (g + 1)] * 0.125  # fold 1/sqrt(DH)
        wk = W_qkv[D + DL * g:D + DL * (g + 1)]
        wv = W_qkv[2 * D + DL * g:2 * D + DL * (g + 1)]
        in_maps.append({
            "xT": np.ascontiguousarray(x[b].T).astype(xnp),
            "wqk": np.ascontiguousarray(np.concatenate([wq, wk], 0).T).astype(wnp),
            "wv": np.ascontiguousarray(wv.T).astype(wnp),
            "wo": np.ascontiguousarray(W_out[:, DL * g:DL * (g + 1)].T).astype(wnp),
            "mask": mask.astype(anp),
        })
    return in_maps


def _assemble(results) -> np.ndarray:
    out = np.empty((B, N, D), np.float32)
    for b in range(B):
        out[b] = (results[2 * b]["oT"].astype(np.float32)
                  + results[2 * b + 1]["oT"].astype(np.float32)).T
    return out


def kernel(x, W_qkv, W_out) -> np.ndarray:
    nc = _get_program()
    in_maps = make_in_maps(x, W_qkv, W_out)
    res = run_bass_kernel_spmd(nc, in_maps, list(range(NCORES)))
    return _assemble(res.results)


def kernel_traced(x, W_qkv, W_out):
    """Like kernel() but with NTFF tracing; returns (out, BassKernelResults)."""
    nc = _get_program()
    in_maps = make_in_maps(x, W_qkv, W_out)
    res = run_bass_kernel_spmd(nc, in_maps, list(range(NCORES)), trace=True)
    return _assemble(res.results), res


def kernel_timed(x, W_qkv, W_out, iters=10, nc=None):
    """Run on HW repeatedly with device-resident inputs; returns
    (out, per_call_seconds_list). Mirrors bass2jax.run_bass_via_pjrt's
    multi-core path but keeps the jitted callable for re-dispatch."""
    import jax
    import numpy as _np
    from jax.sharding import Mesh, PartitionSpec
    from jax.experimental.shard_map import shard_map
    from concourse import bass2jax, mybir as _mb
    import time as _time

    bass2jax.install_neuronx_cc_hook()
    if nc is None:
        nc = _get_program()
    in_maps = make_in_maps(x, W_qkv, W_out)

    part_name = nc.partition_id_tensor.name if nc.partition_id_tensor else None
    in_names, out_names, out_avals, zero_outs = [], [], [], []
    for alloc in nc.m.functions[0].allocations:
        if not isinstance(alloc, _mb.MemoryLocationSet):
            continue
        name = alloc.memorylocations[0].name
        if alloc.kind == "ExternalInput":
            if name != part_name:
                in_names.append(name)
        elif alloc.kind == "ExternalOutput":
            out_names.append(name)
            shape = tuple(alloc.tensor_shape)
            dtype = _mb.dt.np(alloc.dtype)
            out_avals.append(jax.core.ShapedArray(shape, dtype))
            zero_outs.append(_np.zeros(shape, dtype))
    n_params = len(in_names)
    all_names = in_names + out_names
    if part_name is not None:
        all_names = all_names + [part_name]

    def _body(*args):
        operands = list(args)
        if part_name is not None:
            operands.append(bass2jax.partition_id_tensor())
        outs = bass2jax._bass_exec_p.bind(
            *operands,
            out_avals=tuple(out_avals),
            in_names=tuple(all_names),
            out_names=tuple(out_names),
            lowering_input_output_aliases=(),
            sim_require_finite=True,
            sim_require_nnan=True,
            nc=nc,
        )
        return tuple(outs)

    devices = jax.devices()[:NCORES]
    mesh = Mesh(_np.asarray(devices), ("core",))
    nin = n_params + len(out_names)
    fn = jax.jit(
        shard_map(
            _body,
            mesh=mesh,
            in_specs=(PartitionSpec("core"),) * nin,
            out_specs=(PartitionSpec("core"),) * len(out_names),
            check_rep=False,
        ),
        keep_unused=True,
    )
    concat_in = [
        _np.concatenate([_np.asarray(in_maps[c][nm]) for c in range(NCORES)], axis=0)
        for nm in in_names
    ] + [
        _np.zeros((NCORES * z.shape[0], *z.shape[1:]), z.dtype) for z in zero_outs
    ]
    dev_in = [jax.device_put(a) for a in concat_in]
    out = fn(*dev_in)  # compile + warm
    jax.block_until_ready(out)
    times = []
    for _ in range(iters):
        t0 = _time.perf_counter()
        out = fn(*dev_in)
        jax.block_until_ready(out)
        times.append(_time.perf_counter() - t0)
    results = [
        {nm: _np.asarray(out[i]).reshape(NCORES, *out_avals[i].shape)[c]
         for i, nm in enumerate(out_names)}
        for c in range(NCORES)
    ]
    return _assemble(results), times


# revision 21
# speedup vs baseline: 308.8596x; 1.0516x over previous
"""Causal self-attention (B=4, N=2048, D=1024, H=16) on 8 TRN2 NeuronCores.

Sharding: core c handles batch b = c//2 and head group g = c%2 (8 heads,
512 of the 1024 head dims). Each core computes
  qkv projection (its heads) -> causal attention -> partial out-projection
and returns oT_partial = (y_part @ W_out[:, cols].T).T  as [1024, 2048].
Host sums the two head-group partials per batch and transposes back.

All data is staged transposed on the host so every matmul contracts over
the SBUF partition axis:
  qkT = [Wq.T | Wk.T]-proj of xT     (scores need q/k with DH on partitions)
  v   = natural [n, d] layout, with a ones-column appended per head so the
        attention row-sum (softmax denominator) falls out of the same matmul.
Softmax runs without max-subtraction (scores ~ N(0,1), exp is safe in fp32),
masking is a 0/1 multiply on the 4 distinct diagonal-block patterns.
"""

import os
import sys

_TRN_REPO = "/opt/trn_rl_repo"
if _TRN_REPO not in sys.path:
    sys.path.insert(0, _TRN_REPO)

import numpy as np
import ml_dtypes
from contextlib import ExitStack

import concourse.bass as bass
import concourse.bacc as bacc
import concourse.tile as tile
from concourse import mybir
from concourse.bass_utils import run_bass_kernel_spmd

B, N, D, H, DH = 4, 2048, 1024, 16, 64
NCORES = 8
GH = 8          # heads per core
DL = GH * DH    # 512 local head dims
P = 128
CH = 512        # free-dim chunk (one PSUM bank of fp32)
NCH = N // CH   # 4
KT = D // P     # 8 contraction tiles for the projections

F32 = mybir.dt.float32
BF16 = mybir.dt.bfloat16

# dtype knobs (memory formats of the matmul operands)
X_DT = BF16     # xT tiles
W_DT = BF16     # wqk / wv / wo tiles
QK_DT = BF16    # qkT tiles (scores matmul operands)
V_DT = BF16     # v tiles
ATT_DT = BF16   # exp(S.T) tiles / mask
Y_DT = BF16     # yT tiles (out-projection rhs)

USE_GPSIMD_BCAST = True  # rank-1 PE broadcast by default

_NP_DT = {BF16: ml_dtypes.bfloat16, F32: np.float32}


def build_program(reps: int = 1) -> bass.Bass:
    nc = bacc.Bacc("TRN2", target_bir_lowering=False, debug=False)

    xT_d = nc.dram_tensor("xT", [D, N], X_DT, kind="ExternalInput").ap()
    wqk_d = nc.dram_tensor("wqk", [D, 2 * DL], W_DT, kind="ExternalInput").ap()
    wv_d = nc.dram_tensor("wv", [D, DL], W_DT, kind="ExternalInput").ap()
    wo_d = nc.dram_tensor("wo", [DL, D], W_DT, kind="ExternalInput").ap()
    mask_d = nc.dram_tensor("mask", [4 * P, 2 * CH], ATT_DT, kind="ExternalInput").ap()
    oT_d = nc.dram_tensor("oT", [D, N], F32, kind="ExternalOutput").ap()

    with tile.TileContext(nc) as tc:
        for rep in range(reps):
            _emit_rep(nc, tc, rep, xT_d, wqk_d, wv_d, wo_d, mask_d, oT_d)

    nc.compile()
    return nc


def _emit_rep(nc, tc, rep, xT_d, wqk_d, wv_d, wo_d, mask_d, oT_d):
    r = f"_r{rep}"
    with ExitStack() as ctx:
        xt_pool = ctx.enter_context(tc.tile_pool(name="xt" + r, bufs=KT))
        wqk_pool = ctx.enter_context(tc.tile_pool(name="wqk" + r, bufs=KT))
        wv_pool = ctx.enter_context(tc.tile_pool(name="wv" + r, bufs=KT))
        qk_pool = ctx.enter_context(tc.tile_pool(name="qk" + r, bufs=8))
        v_pool = ctx.enter_context(tc.tile_pool(name="v" + r, bufs=N // P))
        mask_pool = ctx.enter_context(tc.tile_pool(name="mask" + r, bufs=4))
        y_pool = ctx.enter_context(tc.tile_pool(name="y" + r, bufs=4))
        wo_pool = ctx.enter_context(tc.tile_pool(name="wo" + r, bufs=4))
        att_pool = ctx.enter_context(tc.tile_pool(name="att" + r, bufs=4))
        nrm_pool = ctx.enter_context(tc.tile_pool(name="nrm" + r, bufs=4))
        osb_pool = ctx.enter_context(tc.tile_pool(name="osb" + r, bufs=4))

        # ---- loads -------------------------------------------------------
        xt = []
        wqkt = []
        wvt = []
        for k in range(KT):
            t = xt_pool.tile([P, N], X_DT, name=f"xt{k}{r}", tag="xt")
            nc.sync.dma_start(t[:], xT_d[k * P:(k + 1) * P, :])
            xt.append(t)
            t = wqk_pool.tile([P, 2 * DL], W_DT, name=f"wqkt{k}{r}", tag="wqkt")
            nc.sync.dma_start(t[:], wqk_d[k * P:(k + 1) * P, :])
            wqkt.append(t)
            t = wv_pool.tile([P, DL], W_DT, name=f"wvt{k}{r}", tag="wvt")
            nc.sync.dma_start(t[:], wv_d[k * P:(k + 1) * P, :])
            wvt.append(t)
        mk = []
        wot = []
        for k in range(4):
            t = mask_pool.tile([P, 2 * CH], ATT_DT, name=f"mk{k}{r}", tag="mk")
            nc.sync.dma_start(t[:], mask_d[k * P:(k + 1) * P, :])
            mk.append(t)
            t = wo_pool.tile([P, D], W_DT, name=f"wot{k}{r}", tag="wot")
            nc.sync.dma_start(t[:], wo_d[k * P:(k + 1) * P, :])
            wot.append(t)

        # ---- pipelined emission: v-proj, then per head-pair
        # (qk-proj -> attention), then out-proj. One shared PSUM pool.
        qkT = [qk_pool.tile([P, N], QK_DT, name=f"qkT{i}{r}", tag="qkT")
               for i in range(8)]
        vsb = [v_pool.tile([P, GH * 65], V_DT, name=f"vsb{i}{r}", tag="vsb")
               for i in range(N // P)]
        yT = [y_pool.tile([P, N], Y_DT, name=f"yT{i}{r}", tag="yT")
              for i in range(4)]

        with tc.tile_pool(name="ps" + r, bufs=1, space="PSUM") as psp:
            # v projection (natural [n, d] layout + ones column per head)
            for mt in range(N // P):
                ps = psp.tile([P, DL], F32, name=f"p1v_{mt}{r}", tag="st", bufs=2)
                for k in range(KT):
                    nc.tensor.matmul(
                        ps[:],
                        lhsT=xt[k][:, mt * P:(mt + 1) * P],
                        rhs=wvt[k][:],
                        start=(k == 0),
                        stop=(k == KT - 1),
                    )
                v3 = vsb[mt].rearrange("p (h c) -> p h c", c=65)
                nc.vector.tensor_copy(
                    v3[:, :, 0:64], ps.rearrange("p (h c) -> p h c", c=64)
                )
                nc.vector.memset(v3[:, :, 64:65], 1.0)

            for hp in range(4):
                # project this pair's q rows (tile hp) and k rows (tile 4+hp)
                for t in (hp, 4 + hp):
                    for j in range(NCH):
                        ps = psp.tile([P, CH], F32, name=f"p1_{t}_{j}{r}",
                                      tag="st", bufs=2)
                        for k in range(KT):
                            nc.tensor.matmul(
                                ps[:],
                                lhsT=wqkt[k][:, t * P:(t + 1) * P],
                                rhs=xt[k][:, j * CH:(j + 1) * CH],
                                start=(k == 0),
                                stop=(k == KT - 1),
                            )
                        nc.vector.tensor_copy(
                            qkT[t][:, j * CH:(j + 1) * CH], ps[:]
                        )

                # causal attention for the pair, row-tiled across the array
                h0, h1 = 2 * hp, 2 * hp + 1
                qt, kt = qkT[hp], qkT[4 + hp]
                for j in range(NCH):
                    ops0 = psp.tile([65, CH], F32, name=f"ops0_{hp}_{j}{r}",
                                    tag="out0", bufs=2)
                    ops1 = psp.tile([65, CH], F32, name=f"ops1_{hp}_{j}{r}",
                                    tag="out1", bufs=2)
                    nmt = 4 * (j + 1)
                    for mt in range(nmt):
                        st = psp.tile([P, 2 * CH], F32, name=f"st_{hp}_{j}_{mt}{r}",
                                      tag="st", bufs=2)
                        # both heads' score blocks concurrently (row strips)
                        nc.tensor.matmul(
                            st[:, 0:CH],
                            lhsT=kt[0:64, mt * P:(mt + 1) * P],
                            rhs=qt[0:64, j * CH:(j + 1) * CH],
                            start=True,
                            stop=True,
                        )
                        nc.tensor.matmul(
                            st[:, CH:2 * CH],
                            lhsT=kt[64:128, mt * P:(mt + 1) * P],
                            rhs=qt[64:128, j * CH:(j + 1) * CH],
                            start=True,
                            stop=True,
                        )
                        at = att_pool.tile([P, 2 * CH], ATT_DT,
                                           name=f"at_{hp}_{j}_{mt}{r}", tag="at")
                        nc.scalar.activation(
                            at[:], st[:], mybir.ActivationFunctionType.Exp
                        )
                        ko = mt - 4 * j
                        if ko >= 0:  # diagonal partial block (same mask, both heads)
                            atm = att_pool.tile([P, 2 * CH], ATT_DT,
                                                name=f"atm_{hp}_{j}_{mt}{r}",
                                                tag="at")
                            nc.vector.tensor_mul(atm[:], at[:], mk[ko][:])
                            at = atm
                        nc.tensor.matmul(
                            ops0[:],
                            lhsT=vsb[mt][:, 65 * h0:65 * h0 + 65],
                            rhs=at[:, 0:CH],
                            start=(mt == 0),
                            stop=(mt == nmt - 1),
                        )
                        nc.tensor.matmul(
                            ops1[:],
                            lhsT=vsb[mt][:, 65 * h1:65 * h1 + 65],
                            rhs=at[:, CH:2 * CH],
                            start=(mt == 0),
                            stop=(mt == nmt - 1),
                        )
                    # normalize while copying out of PSUM:
                    # yT rows = ops[0:64] * bcast(1 / ops[64])
                    jc = slice(j * CH, (j + 1) * CH)
                    for ii, ops in ((0, ops0), (1, ops1)):
                        dn = nrm_pool.tile([1, CH], F32,
                                           name=f"dn{ii}_{hp}_{j}{r}", tag="dn",
                                           bufs=4)
                        nc.vector.tensor_copy(dn[:], ops[64:65, :])
                        rc = nrm_pool.tile([1, CH], F32,
                                           name=f"rc{ii}_{hp}_{j}{r}", tag="rc",
                                           bufs=4)
                        nc.vector.reciprocal_approx_fast(out=rc[:], in_=dn[:])
                        bc = nrm_pool.tile([64, CH], F32,
                                           name=f"bc{ii}_{hp}_{j}{r}", tag="bc",
                                           bufs=4)
                        nc.gpsimd.partition_broadcast(bc[:], rc[:])
                        nc.vector.tensor_mul(
                            yT[hp][64 * ii:64 * ii + 64, jc],
                            ops[0:64, :],
                            bc[:],
                        )

            # out projection: oT = (yT.T @ woT).T
            for e in range(8):
                for j in range(NCH):
                    ps = psp.tile([P, CH], F32, name=f"p3_{e}_{j}{r}",
                                  tag="st", bufs=2)
                    for d4 in range(4):
                        nc.tensor.matmul(
                            ps[:],
                            lhsT=wot[d4][:, e * P:(e + 1) * P],
                            rhs=yT[d4][:, j * CH:(j + 1) * CH],
                            start=(d4 == 0),
                            stop=(d4 == 3),
                        )
                    ob = osb_pool.tile([P, CH], F32, name=f"ob_{e}_{j}{r}", tag="ob")
                    nc.vector.tensor_copy(ob[:], ps[:])
                    nc.sync.dma_start(
                        oT_d[e * P:(e + 1) * P, j * CH:(j + 1) * CH], ob[:]
                    )


_PROGRAM = None


def _get_program() -> bass.Bass:
    global _PROGRAM
    if _PROGRAM is None:
        _PROGRAM = build_program()
    return _PROGRAM


def _timing_program(reps: int) -> bass.Bass:
    return build_program(reps=reps)


def make_mask() -> np.ndarray:
    p = np.arange(P)[:, None]
    f = np.arange(CH)[None, :]
    m = np.zeros((4 * P, CH), np.float32)
    for k in range(4):
        m[k * P:(k + 1) * P] = (p + P * k <= f)
    return np.tile(m, (1, 2))


def make_in_maps(x, W_qkv, W_out):
    x = np.asarray(x, np.float32)
    W_qkv = np.asarray(W_qkv, np.float32)
    W_out = np.asarray(W_out, np.float32)
    mask = make_mask()
    xnp, wnp, anp = _NP_DT[X_DT], _NP_DT[W_DT], _NP_DT[ATT_DT]
    in_maps = []
    for c in range(NCORES):
        b, g = divmod(c, 2)
        wq = W_qkv[DL * g:DL * (g + 1)] * 0.125  # fold 1/sqrt(DH)
        wk = W_qkv[D + DL * g:D + DL * (g + 1)]
        wv = W_qkv[2 * D + DL * g:2 * D + DL * (g + 1)]
        in_maps.append({
            "xT": np.ascontiguousarray(x[b].T).astype(xnp),
            "wqk": np.ascontiguousarray(np.concatenate([wq, wk], 0).T).astype(wnp),
            "wv": np.ascontiguousarray(wv.T).astype(wnp),
            "wo": np.ascontiguousarray(W_out[:, DL * g:DL * (g + 1)].T).astype(wnp),
            "mask": mask.astype(anp),
        })
    return in_maps


def _assemble(results) -> np.ndarray:
    out = np.empty((B, N, D), np.float32)
    for b in range(B):
        out[b] = (results[2 * b]["oT"].astype(np.float32)
                  + results[2 * b + 1]["oT"].astype(np.float32)).T
    return out


def kernel(x, W_qkv, W_out) -> np.ndarray:
    nc = _get_program()
    in_maps = make_in_maps(x, W_qkv, W_out)
    res = run_bass_kernel_spmd(nc, in_maps, list(range(NCORES)))
    return _assemble(res.results)


def kernel_traced(x, W_qkv, W_out):
    """Like kernel() but with NTFF tracing; returns (out, BassKernelResults)."""
    nc = _get_program()
    in_maps = make_in_maps(x, W_qkv, W_out)
    res = run_bass_kernel_spmd(nc, in_maps, list(range(NCORES)), trace=True)
    return _assemble(res.results), res


def kernel_timed(x, W_qkv, W_out, iters=10, nc=None):
    """Run on HW repeatedly with device-resident inputs; returns
    (out, per_call_seconds_list). Mirrors bass2jax.run_bass_via_pjrt's
    multi-core path but keeps the jitted callable for re-dispatch."""
    import jax
    import numpy as _np
    from jax.sharding import Mesh, PartitionSpec
    from jax.experimental.shard_map import shard_map
    from concourse import bass2jax, mybir as _mb
    import time as _time

    bass2jax.install_neuronx_cc_hook()
    if nc is None:
        nc = _get_program()
    in_maps = make_in_maps(x, W_qkv, W_out)

    part_name = nc.partition_id_tensor.name if nc.partition_id_tensor else None
    in_names, out_names, out_avals, zero_outs = [], [], [], []
    for alloc in nc.m.functions[0].allocations:
        if not isinstance(alloc, _mb.MemoryLocationSet):
            continue
        name = alloc.memorylocations[0].name
        if alloc.kind == "ExternalInput":
            if name != part_name:
                in_names.append(name)
        elif alloc.kind == "ExternalOutput":
            out_names.append(name)
            shape = tuple(alloc.tensor_shape)
            dtype = _mb.dt.np(alloc.dtype)
            out_avals.append(jax.core.ShapedArray(shape, dtype))
            zero_outs.append(_np.zeros(shape, dtype))
    n_params = len(in_names)
    all_names = in_names + out_names
    if part_name is not None:
        all_names = all_names + [part_name]

    def _body(*args):
        operands = list(args)
        if part_name is not None:
            operands.append(bass2jax.partition_id_tensor())
        outs = bass2jax._bass_exec_p.bind(
            *operands,
            out_avals=tuple(out_avals),
            in_names=tuple(all_names),
            out_names=tuple(out_names),
            lowering_input_output_aliases=(),
            sim_require_finite=True,
            sim_require_nnan=True,
            nc=nc,
        )
        return tuple(outs)

    devices = jax.devices()[:NCORES]
    mesh = Mesh(_np.asarray(devices), ("core",))
    nin = n_params + len(out_names)
    fn = jax.jit(
        shard_map(
            _body,
            mesh=mesh,
            in_specs=(PartitionSpec("core"),) * nin,
            out_specs=(PartitionSpec("core"),) * len(out_names),
            check_rep=False,
        ),
        keep_unused=True,
    )
    concat_in = [
        _np.concatenate([_np.asarray(in_maps[c][nm]) for c in range(NCORES)], axis=0)
        for nm in in_names
    ] + [
        _np.zeros((NCORES * z.shape[0], *z.shape[1:]), z.dtype) for z in zero_outs
    ]
    dev_in = [jax.device_put(a) for a in concat_in]
    out = fn(*dev_in)  # compile + warm
    jax.block_until_ready(out)
    times = []
    for _ in range(iters):
        t0 = _time.perf_counter()
        out = fn(*dev_in)
        jax.block_until_ready(out)
        times.append(_time.perf_counter() - t0)
    results = [
        {nm: _np.asarray(out[i]).reshape(NCORES, *out_avals[i].shape)[c]
         for i, nm in enumerate(out_names)}
        for c in range(NCORES)
    ]
    return _assemble(results), times
